# revision 28
# baseline (speedup 1.0000x reference)
"""Llama GQA attention (B=2,S=2048,H=32,KV=8,D=128,DM=4096) on 8 trn2 cores.

Sharding: DP=2 over sequences x TP=4 over heads. Core c = (b=c//4, g=c%4):
seq b's 2048 tokens, q-heads [8g,8g+8), kv-heads [2g,2g+2). Each core computes
its partial o-proj output; host sums the 4 TP partials per sequence.

Device layout trick: everything lives transposed ([feat, tok]) so the
contraction dim is always on partitions and no on-chip transposes are needed.
  qkv^T = W^T.T @ hidden^T          (W^T, hidden^T pre-transposed on host)
  S^T[j,i] = (k^T).T @ q^T          (contraction d=128 = one partition tile)
  P^T = exp(scale*S^T) * mask       (no max-subtraction: scores ~ N(0,1))
  C^T accumulated as two 65-row matmuls with ones-augmented V halves, so the
  65th psum row is the softmax denominator l (no separate ones-matmul).
  1/l broadcast across partitions on GpSimd; out^T = Wo^T.T @ (C^T / l).
RoPE: rotate-half via partition-offset vector muls with host-negated sin.
All matmuls bf16 inputs, fp32 PSUM accumulation.
"""

import numpy as np
import ml_dtypes

import concourse.bass as bass
import concourse.mybir as mybir
import concourse.tile as tile
from concourse.bass_utils import run_bass_kernel_spmd

F32 = mybir.dt.float32
BF16 = mybir.dt.bfloat16
BF = ml_dtypes.bfloat16


class Cfg:
    def __init__(self, S=2048, H=32, KV=8, D=128, TP=4, DP=2, TB=512, IB=512):
        self.S, self.H, self.KV, self.D = S, H, KV, D
        self.TP, self.DP = TP, DP
        self.DM = H * D
        self.HL = H // TP            # local q heads
        self.KVL = KV // TP          # local kv heads
        self.QF = self.HL * D        # local q feats
        self.KF = self.KVL * D
        self.VF = self.KVL * D
        self.LF = self.HL * D        # local o-proj contraction feats
        self.NKT = self.DM // 128    # K-tiles for qkv proj
        self.NQK = (self.QF + self.KF) // 128
        self.TB = min(TB, S)         # token block (qkv / o-proj moving dim)
        self.IB = min(IB, S)         # query block in attention
        self.ND = self.IB // 128     # j-tiles per i-block (diag patterns)
        self.GJ = 2 if self.ND >= 2 else 1   # j-tiles per exp group
        self.scale = float(D) ** -0.5


def build_kernel(tc, cfg):
    nc = tc.nc
    S, D = cfg.S, cfg.D
    TB, IB, ND, GJ = cfg.TB, cfg.IB, cfg.ND, cfg.GJ
    NKT, NQK = cfg.NKT, cfg.NQK
    NTB = S // TB
    NTT = TB // 128                  # tok tiles per block (for V)
    NIB = S // IB
    NKH = 16                         # hid k-tiles per half (A: 0..15, B: 16..31)

    hid = nc.dram_tensor("hid_t", [cfg.DM, S], BF16, kind="ExternalInput").ap()
    wqk = nc.dram_tensor("wqk_t", [cfg.DM, cfg.QF + cfg.KF], BF16, kind="ExternalInput").ap()
    wv = nc.dram_tensor("wv_t", [cfg.DM, cfg.VF], BF16, kind="ExternalInput").ap()
    wo = nc.dram_tensor("wo_t", [cfg.LF, cfg.DM], BF16, kind="ExternalInput").ap()
    cos = nc.dram_tensor("cos_t", [128, S], BF16, kind="ExternalInput").ap()
    sin = nc.dram_tensor("sin_t", [128, S], BF16, kind="ExternalInput").ap()
    msk = nc.dram_tensor("masks", [128, 128], BF16, kind="ExternalInput").ap()
    out = nc.dram_tensor("out_t", [cfg.DM, S], BF16, kind="ExternalOutput").ap()

    hid_r = hid.rearrange("(a p) t -> p a t", p=128)
    wqk_r = wqk.rearrange("(a p) f -> p a f", p=128)
    wv_r = wv.rearrange("(a p) f -> p a f", p=128)
    wo_r = wo.rearrange("(a p) f -> p a f", p=128)

    with tc.tile_pool(name="res", bufs=1) as res:
        qkT = res.tile([128, NQK, S], BF16, tag="qkT")
        # V with interleaved ones columns: per kv head 130 cols =
        # [64 dims][1s][64 dims][1s]
        v_sb = res.tile([128, S // 128, cfg.KVL * 130], BF16, tag="v")
        attnT = res.tile([128, cfg.HL, S], BF16, tag="attnT")
        cos_t = res.tile([128, S], BF16, tag="cos")
        sin_t = res.tile([128, S], BF16, tag="sin")
        msk_t = res.tile([128, 128], BF16, tag="msk")

        # constants ride behind the first hid-block loads (issued in phase 1);
        # cos/sin are first needed ~8us in, masks/wo much later
        NKF = cfg.LF // 128
        wo_pre = res.tile([128, 2, NKF, 128], BF16, tag="wo_pre")
        # ones columns of v_sb (cols 64 and 129 of each 130-col kv block)
        v_onr = v_sb[:].rearrange("p t (kv h c) -> p t kv h c", kv=cfg.KVL, h=2)
        nc.vector.memset(v_onr[:, :, :, :, 64:65], 1.0)

        # wv is small: load whole tensor once (scalar queue: gpsimd queue must
        # reach the first wqk tile DMA immediately)
        wv_t = res.tile([128, NKT, cfg.VF], BF16, tag="wv")
        nc.scalar.dma_start(wv_t[:], wv_r[:])

        # warm the scalar engine's Exp table before phase 2 needs it
        warm = res.tile([1, 8], F32, tag="warm")
        nc.vector.memset(warm[:], 0.0)
        nc.scalar.activation(warm[:], warm[:], mybir.ActivationFunctionType.Exp)

        # ---------------- Phase 1: fused QKV projection + RoPE ----------------
        with tc.tile_pool(name="p1", bufs=2) as p1, \
             tc.tile_pool(name="p1hA", bufs=2) as p1hA, \
             tc.tile_pool(name="p1hB", bufs=1) as p1hB, \
             tc.tile_pool(name="p1w", bufs=3) as p1w, \
             tc.tile_pool(name="ps_qk", bufs=3, space="PSUM") as ps_qk, \
             tc.tile_pool(name="ps_v", bufs=2, space="PSUM") as ps_v:
            for tb in range(NTB):
                ts = slice(tb * TB, (tb + 1) * TB)
                hbA = p1hA.tile([128, NKH, TB], BF16, tag="hbA")
                hbB = p1hB.tile([128, NKH, TB], BF16, tag="hbB")
                nc.sync.dma_start(hbA[:], hid_r[:, 0:NKH, ts])
                nc.sync.dma_start(hbB[:], hid_r[:, NKH:NKT, ts])
                if tb == 0:
                    nc.sync.dma_start(cos_t[:], cos[:])
                    nc.sync.dma_start(sin_t[:], sin[:])
                    nc.sync.dma_start(msk_t[:], msk[:])
                    nc.sync.dma_start(wo_pre[:, 0], wo_r[:, :, 0:128])
                    nc.sync.dma_start(wo_pre[:, 1], wo_r[:, :, 128:256])

                def hb(kk):
                    return hbA[:, kk, :] if kk < NKH else hbB[:, kk - NKH, :]

                # k feat tiles first so attention's first scores unblock early
                for ft in list(range(cfg.HL, NQK)) + list(range(cfg.HL)):
                    wt = p1w.tile([128, NKT, 128], BF16, tag="wt")
                    nc.gpsimd.dma_start(wt[:], wqk_r[:, :, ft * 128:(ft + 1) * 128])
                    ps = ps_qk.tile([128, TB], F32, tag="ps")
                    for kk in range(NKT):
                        nc.tensor.matmul(ps[:], wt[:, kk, :], hb(kk),
                                         start=(kk == 0), stop=(kk == NKT - 1))
                    # RoPE: qkT = ps*cos + rotate_half(ps)*sin, with the
                    # rotate done by partition-offset reads and the sign
                    # folded into sin (host negates rows 0:64)
                    t1 = p1.tile([128, TB], F32, tag="t1")
                    nc.vector.tensor_mul(t1[:], ps[:], cos_t[:, ts])
                    t2 = p1.tile([128, TB], F32, tag="t2")
                    nc.vector.tensor_mul(t2[0:64, :], ps[64:128, :], sin_t[0:64, ts])
                    nc.vector.tensor_mul(t2[64:128, :], ps[0:64, :], sin_t[64:128, ts])
                    nc.vector.tensor_add(qkT[:, ft, ts], t1[:], t2[:])
                for tt in range(NTT):
                    psv = ps_v.tile([128, cfg.VF], F32, tag="psv")
                    for kk in range(NKT):
                        nc.tensor.matmul(psv[:], hb(kk)[:, tt * 128:(tt + 1) * 128],
                                         wv_t[:, kk, :],
                                         start=(kk == 0), stop=(kk == NKT - 1))
                    vd = v_sb[:, tb * NTT + tt, :].rearrange(
                        "p (kv h c) -> p kv h c", kv=cfg.KVL, h=2)[:, :, :, 0:64]
                    nc.scalar.copy(
                        vd, psv[:].rearrange("p (kv h d) -> p kv h d", kv=cfg.KVL, h=2))

        # ---------------- Phase 2: causal GQA attention ----------------
        with tc.tile_pool(name="p2", bufs=2) as p2, \
             tc.tile_pool(name="ps_s", bufs=2, space="PSUM") as ps_s, \
             tc.tile_pool(name="ps_cA", bufs=2, space="PSUM") as ps_cA, \
             tc.tile_pool(name="ps_cB", bufs=2, space="PSUM") as ps_cB:
            for h in range(cfg.HL):
                ftk = cfg.HL + (h // (cfg.HL // cfg.KVL))  # k feat-tile for this head
                hkv = h // (cfg.HL // cfg.KVL)
                va_all = v_sb[:].rearrange("p t (kv c) -> p t kv c", kv=cfg.KVL)
                for ib in reversed(range(NIB)):
                    isl = slice(ib * IB, (ib + 1) * IB)
                    njt = ND * (ib + 1)
                    cpsA = ps_cA.tile([65, IB], F32, tag="cpsA")
                    cpsB = ps_cB.tile([65, IB], F32, tag="cpsB")
                    first_diag = njt - ND
                    for jg in range((njt + GJ - 1) // GJ):
                        ngj = min(GJ, njt - jg * GJ)
                        # for diagonal tiles skip the fully-masked columns
                        # left of the diagonal: tile jj covers keys from
                        # col 128*(jj-first_diag) of this i-block onward
                        cs = [max(0, 128 * (jg * GJ + jl - first_diag))
                              for jl in range(ngj)]
                        sps = ps_s.tile([128, GJ, IB], F32, tag="sps")
                        for jl in range(ngj):
                            jj = jg * GJ + jl
                            nc.tensor.matmul(
                                sps[:, jl, cs[jl]:],
                                qkT[:, ftk, jj * 128:(jj + 1) * 128],
                                qkT[:, h, ib * IB + cs[jl]:(ib + 1) * IB],
                                start=True, stop=True)
                        pt = p2.tile([128, GJ, IB], BF16, tag="pt")
                        if cs[-1] == 0:  # no diagonal in group: one wide exp
                            nc.scalar.activation(
                                pt[:, 0:ngj, :], sps[:, 0:ngj, :],
                                mybir.ActivationFunctionType.Exp, scale=cfg.scale)
                        else:
                            for jl in range(ngj):
                                nc.scalar.activation(
                                    pt[:, jl, cs[jl]:], sps[:, jl, cs[jl]:],
                                    mybir.ActivationFunctionType.Exp, scale=cfg.scale)
                                nc.vector.tensor_mul(
                                    pt[:, jl, cs[jl]:cs[jl] + 128],
                                    pt[:, jl, cs[jl]:cs[jl] + 128], msk_t[:])
                        for cps, c0 in ((cpsA, 0), (cpsB, 65)):
                            for jl in range(ngj):
                                jj = jg * GJ + jl
                                nc.tensor.matmul(
                                    cps[:, cs[jl]:], va_all[:, jj, hkv, c0:c0 + 65],
                                    pt[:, jl, cs[jl]:],
                                    start=(jj == 0), stop=(jj == njt - 1),
                                    skip_group_check=True)
                    lrow = p2.tile([1, IB], F32, tag="lrow")
                    nc.vector.tensor_copy(lrow[:], cpsA[64:65, :])
                    rec = p2.tile([1, IB], F32, tag="rec")
                    nc.vector.reciprocal_approx_fast(rec[:], lrow[:])
                    rb = p2.tile([128, IB], F32, tag="rb")
                    nc.gpsimd.partition_broadcast(rb[:], rec[:])
                    nc.vector.tensor_mul(attnT[0:64, h, isl], cpsA[0:64, :], rb[0:64, :])
                    nc.vector.tensor_mul(attnT[64:128, h, isl], cpsB[0:64, :], rb[64:128, :])

        # ---------------- Phase 3: o-proj (partial; host all-reduces) ----------------
        with tc.tile_pool(name="p3", bufs=2) as p3, \
             tc.tile_pool(name="p3w", bufs=2) as p3w, \
             tc.tile_pool(name="ps_o", bufs=4, space="PSUM") as ps_o:
            NOF = cfg.DM // 128
            for of in range(NOF):
                if of < 2:
                    wt = wo_pre[:, of]
                else:
                    wt = p3w.tile([128, NKF, 128], BF16, tag="wot")
                    nc.gpsimd.dma_start(wt[:], wo_r[:, :, of * 128:(of + 1) * 128])
                o_sb = p3.tile([128, S], BF16, tag="o_sb")
                for tb in range(NTB):
                    ts = slice(tb * TB, (tb + 1) * TB)
                    ps = ps_o.tile([128, TB], F32, tag="pso")
                    for kf in range(NKF):
                        nc.tensor.matmul(ps[:], wt[:, kf, :], attnT[:, kf, ts],
                                         start=(kf == 0), stop=(kf == NKF - 1))
                    nc.scalar.copy(o_sb[:, ts], ps[:])
                nc.gpsimd.dma_start(out[of * 128:(of + 1) * 128, :], o_sb[:])


def shard_inputs(hidden_states, cos, sin, qkv_weight, o_weight, cfg):
    """Host-side shard + transpose + bf16 cast. Returns list of 8 in_maps."""
    S, D, HL, KVL = cfg.S, cfg.D, cfg.HL, cfg.KVL
    H, KV = cfg.H, cfg.KV
    # RoPE tables (identical for both sequences - positions restart).
    # sin rows 0:64 carry the rotate-half sign flip.
    cos_t = np.ascontiguousarray(cos[:S].T).astype(BF)
    sin_np = np.asarray(sin[:S].T, dtype=np.float32).copy()
    sin_np[0:64] = -sin_np[0:64]
    sin_t = sin_np.astype(BF)
    # single triangular diag-block mask [128, 128]: col >= row
    j = np.arange(128)[:, None]
    i = np.arange(128)[None, :]
    masks = (i >= j).astype(BF)

    in_maps = []
    for core in range(8):
        b, g = core // cfg.TP, core % cfg.TP
        tok = slice(b * S, (b + 1) * S)
        qr = slice(g * HL * D, (g + 1) * HL * D)
        kr = slice(H * D + g * KVL * D, H * D + (g + 1) * KVL * D)
        vr = slice((H + KV) * D + g * KVL * D, (H + KV) * D + (g + 1) * KVL * D)
        wqk_t = np.ascontiguousarray(
            np.concatenate([qkv_weight[qr], qkv_weight[kr]], 0).T).astype(BF)
        wv_t = np.ascontiguousarray(qkv_weight[vr].T).astype(BF)
        wo_t = np.ascontiguousarray(o_weight[:, qr].T).astype(BF)
        hid_t = np.ascontiguousarray(hidden_states[tok].T).astype(BF)
        in_maps.append({
            "hid_t": hid_t, "wqk_t": wqk_t, "wv_t": wv_t, "wo_t": wo_t,
            "cos_t": cos_t, "sin_t": sin_t, "masks": masks,
        })
    return in_maps


def unshard(results, cfg):
    T = cfg.DP * cfg.S
    out = np.zeros((T, cfg.DM), np.float32)
    for core, r in enumerate(results):
        b = core // cfg.TP
        out[b * cfg.S:(b + 1) * cfg.S] += np.asarray(r["out_t"]).T.astype(np.float32)
    return out.reshape(1, T, cfg.DM)


def _run(inputs, cfg, trace=False):
    import concourse.bacc as bacc
    nc = bacc.Bacc("TRN2", target_bir_lowering=False, debug=False,
                   enable_asserts=False, num_devices=8)
    with tile.TileContext(nc) as tc:
        build_kernel(tc, cfg)
    nc.compile()
    in_maps = shard_inputs(**inputs, cfg=cfg)
    res = run_bass_kernel_spmd(nc, in_maps, core_ids=list(range(8)), trace=trace)
    return unshard(res.results, cfg), res


def kernel(**inputs):
    out, _ = _run(inputs, Cfg())
    return out
